# revision 4
# baseline (speedup 1.0000x reference)
"""Distributed attention head kernel for 8 TRN2 NeuronCores.

Problem: Q,K,V [4,4096,64] f32; out = softmax(Q K^T / 8) V  per batch.

Sharding (data parallel, zero collectives): core c handles batch b=c//2,
query half h=c%2 -> per-core Q [2048,64], K/V full [4096,64] for that batch.

Per-core algorithm (transposed-scores formulation so probs never need an
on-chip transpose before the PV matmul):
  - QT [64,2048], KT [64,4096] built with TensorE transposes.
  - V loaded as 32 tiles [128,65] with a ones column (65th col) so the
    softmax denominator falls out of the PV matmul for free.
  - for each q-half (1024 q) x 32 k-tiles:
      scoresT[k=128, q=1024] = KT_tile^T-matmul in PSUM (f32r, 2 MMs N=512)
      probsT = exp(scale * scoresT)  (ScalarE, 1 call, PSUM->SBUF)
      outT'[65, 1024] += V'_tile^T @ probsT   (accumulate in PSUM)
  - epilogue: reciprocal of denominator row, TensorE-transpose back to
    [128,65], per-partition scalar multiply, DMA out.
"""

import os
import sys
from contextlib import ExitStack

import numpy as np

sys.path.insert(0, "/opt/trn_rl_repo")

import concourse.bass as bass  # noqa: E402
import concourse.tile as tile  # noqa: E402
from concourse import bacc, masks, mybir  # noqa: E402
from concourse.bass_utils import run_bass_kernel_spmd  # noqa: E402

B, S, D = 4, 4096, 64
N_CORES = 8
SQ = B * S // N_CORES  # 2048 queries per core
SCALE = 1.0 / np.sqrt(np.float32(D))  # 0.125

F32 = mybir.dt.float32
# f32r: fp32-stored operands streamed through the PE at bf16 rate (1 cyc/row
# for N>=256) instead of fp32's 4 cyc/row. Set MM_DT = F32 for full-precision.
F32R = mybir.dt.float32r
MM_DT = F32R

N_KT = S // 128          # 32 k-tiles
N_QH = 2                 # q halves per core
QH = SQ // N_QH          # 1024
N_QT = SQ // 128         # 16 q-tiles (for transposing Q)


def _mm(t):
    """View an f32 AP with the matmul streaming dtype."""
    return t.bitcast(MM_DT) if MM_DT is not F32 else t


def _emit(ctx: ExitStack, tc: "tile.TileContext", q_ext, k_ext, v_ext, out_ext):
    nc = tc.nc

    const_pool = ctx.enter_context(tc.tile_pool(name="const", bufs=1))
    qin_pool = ctx.enter_context(tc.tile_pool(name="qin", bufs=4))
    kin_pool = ctx.enter_context(tc.tile_pool(name="kin", bufs=4))
    qt_pool = ctx.enter_context(tc.tile_pool(name="qt", bufs=1))
    kt_pool = ctx.enter_context(tc.tile_pool(name="kt", bufs=1))
    v_pool = ctx.enter_context(tc.tile_pool(name="vv", bufs=1))
    probs_pool = ctx.enter_context(tc.tile_pool(name="probs", bufs=3))
    outsb_pool = ctx.enter_context(tc.tile_pool(name="outsb", bufs=2))
    res_pool = ctx.enter_context(tc.tile_pool(name="res", bufs=3))
    rcol_pool = ctx.enter_context(tc.tile_pool(name="rcol", bufs=3))

    # PSUM: 8 banks x 2KiB. tpsum 2x[128,128] = 2 banks; scores 2x[128,1024]
    # = 4 banks; outT' accumulator [65,1024] = 2 banks. Total 8.
    tpsum_pool = ctx.enter_context(tc.tile_pool(name="tpsum", bufs=2, space="PSUM"))
    spsum_pool = ctx.enter_context(tc.tile_pool(name="spsum", bufs=2, space="PSUM"))
    opsum_pool = ctx.enter_context(tc.tile_pool(name="opsum", bufs=1, space="PSUM"))

    identity = const_pool.tile([128, 128], F32)
    masks.make_identity(nc, identity[:])

    # --- Q^T [64, SQ] ---
    qt = qt_pool.tile([64, SQ], MM_DT)
    for t in range(N_QT):
        qin = qin_pool.tile([128, D], F32)
        nc.sync.dma_start(qin[:], q_ext[t * 128:(t + 1) * 128, :])
        pt = tpsum_pool.tile([128, 128], F32, tag="tp")
        nc.tensor.transpose(pt[:64, :128], qin[:], identity[:])
        nc.vector.tensor_copy(qt[:, t * 128:(t + 1) * 128], pt[:64, :128])

    # --- K^T [64, S] ---
    kt = kt_pool.tile([64, S], MM_DT)
    for i in range(N_KT):
        kin = kin_pool.tile([128, D], F32)
        nc.sync.dma_start(kin[:], k_ext[i * 128:(i + 1) * 128, :])
        pt = tpsum_pool.tile([128, 128], F32, tag="tp")
        nc.tensor.transpose(pt[:64, :128], kin[:], identity[:])
        nc.vector.tensor_copy(kt[:, i * 128:(i + 1) * 128], pt[:64, :128])

    # --- V' tiles [128, 65] with ones column ---
    # DMA lands in an f32 staging tile; a DVE copy performs the f32->f32r
    # rounding walrus requires for matmul operands.
    v_tiles = []
    for i in range(N_KT):
        vstage = kin_pool.tile([128, 65], F32, tag="vstage")
        nc.gpsimd.memset(vstage[:, 64:65], 1.0)
        nc.sync.dma_start(vstage[:, 0:64], v_ext[i * 128:(i + 1) * 128, :])
        vt = v_pool.tile([128, 65], MM_DT, tag=f"v{i}")
        nc.vector.tensor_copy(vt[:, :], vstage[:, :])
        v_tiles.append(vt)

    # --- main loop ---
    for h in range(N_QH):
        q0 = h * QH
        outp = opsum_pool.tile([65, QH], F32)
        for i in range(N_KT):
            sp = spsum_pool.tile([128, QH], F32)
            for j in range(QH // 512):
                nc.tensor.matmul(
                    sp[:, j * 512:(j + 1) * 512],
                    kt[:, i * 128:(i + 1) * 128],
                    qt[:, q0 + j * 512:q0 + (j + 1) * 512],
                    start=True,
                    stop=True,
                )
            pb = probs_pool.tile([128, QH], MM_DT)
            nc.scalar.activation(
                pb[:], sp[:], mybir.ActivationFunctionType.Exp, scale=float(SCALE)
            )
            for j in range(QH // 512):
                nc.tensor.matmul(
                    outp[:, j * 512:(j + 1) * 512],
                    v_tiles[i][:, :],
                    pb[:, j * 512:(j + 1) * 512],
                    start=(i == 0),
                    stop=(i == N_KT - 1),
                )
        # epilogue for this q-half
        osb = outsb_pool.tile([65, QH], F32)
        nc.vector.tensor_copy(osb[:], outp[:])
        nc.vector.reciprocal(osb[64:65, :], osb[64:65, :])
        for j in range(QH // 128):
            pt = tpsum_pool.tile([128, 128], F32, tag="tp")
            nc.tensor.transpose(
                pt[:128, :65], osb[:, j * 128:(j + 1) * 128], identity[:65, :65]
            )
            rcol = rcol_pool.tile([128, 1], F32)
            nc.vector.tensor_copy(rcol[:], pt[:, 64:65])
            res = res_pool.tile([128, 64], F32)
            nc.vector.tensor_scalar_mul(res[:], pt[:, 0:64], rcol[:])
            nc.sync.dma_start(
                out_ext[q0 + j * 128:q0 + (j + 1) * 128, :], res[:]
            )


_CACHED_NC = None
LAST_RESULT = None


def _build():
    global _CACHED_NC
    if _CACHED_NC is not None:
        return _CACHED_NC
    nc = bacc.Bacc("TRN2", target_bir_lowering=False, debug=False, num_devices=N_CORES)
    q_ext = nc.declare_dram_parameter("Q", [SQ, D], F32, isOutput=False)
    k_ext = nc.declare_dram_parameter("K", [S, D], F32, isOutput=False)
    v_ext = nc.declare_dram_parameter("V", [S, D], F32, isOutput=False)
    out_ext = nc.declare_dram_parameter("out", [SQ, D], F32, isOutput=True)
    with tile.TileContext(nc) as tc:
        with ExitStack() as ctx:
            _emit(ctx, tc, q_ext.ap(), k_ext.ap(), v_ext.ap(), out_ext.ap())
    nc.compile()
    _CACHED_NC = nc
    return nc


def kernel(Q: np.ndarray, K: np.ndarray, V: np.ndarray) -> np.ndarray:
    global LAST_RESULT
    Q = np.ascontiguousarray(np.asarray(Q, dtype=np.float32))
    K = np.ascontiguousarray(np.asarray(K, dtype=np.float32))
    V = np.ascontiguousarray(np.asarray(V, dtype=np.float32))
    assert Q.shape == (B, S, D) and K.shape == (B, S, D) and V.shape == (B, S, D)

    nc = _build()
    in_maps = []
    for c in range(N_CORES):
        b, h = divmod(c, N_QH)
        in_maps.append(
            {
                "Q": np.ascontiguousarray(Q[b, h * SQ:(h + 1) * SQ, :]),
                "K": K[b],
                "V": V[b],
            }
        )
    trace = bool(os.environ.get("KERNEL_TRACE"))
    LAST_RESULT = run_bass_kernel_spmd(
        nc, in_maps, core_ids=list(range(N_CORES)), trace=trace
    )
    out = np.empty((B, S, D), dtype=np.float32)
    for c in range(N_CORES):
        b, h = divmod(c, N_QH)
        out[b, h * SQ:(h + 1) * SQ, :] = LAST_RESULT.results[c]["out"]
    return out


# revision 6
# speedup vs baseline: 1.2544x; 1.2544x over previous
"""Distributed attention head kernel for 8 TRN2 NeuronCores.

Problem: Q,K,V [4,4096,64] f32; out = softmax(Q K^T / 8) V  per batch.

Sharding (data parallel, zero collectives): core c handles batch b=c//2,
query half h=c%2 -> per-core Q [2048,64], K/V full [4096,64] for that batch.

Per-core algorithm (transposed-scores formulation so probs never need an
on-chip transpose before the PV matmul):
  - QT [64,2048], KT [64,4096] built with TensorE transposes.
  - V loaded as 32 tiles [128,65] with a ones column (65th col) so the
    softmax denominator falls out of the PV matmul for free.
  - for each q-half (1024 q) x 32 k-tiles:
      scoresT[k=128, q=1024] = KT_tile^T-matmul in PSUM (f32r, 2 MMs N=512)
      probsT = exp(scale * scoresT)  (ScalarE, 1 call, PSUM->SBUF)
      outT'[65, 1024] += V'_tile^T @ probsT   (accumulate in PSUM)
  - epilogue: reciprocal of denominator row, TensorE-transpose back to
    [128,65], per-partition scalar multiply, DMA out.
"""

import os
import sys
from contextlib import ExitStack

import numpy as np

sys.path.insert(0, "/opt/trn_rl_repo")

import concourse.bass as bass  # noqa: E402
import concourse.tile as tile  # noqa: E402
from concourse import bacc, masks, mybir  # noqa: E402
from concourse.bass_utils import run_bass_kernel_spmd  # noqa: E402

B, S, D = 4, 4096, 64
N_CORES = 8
SQ = B * S // N_CORES  # 2048 queries per core
SCALE = 1.0 / np.sqrt(np.float32(D))  # 0.125

F32 = mybir.dt.float32
# bf16 matmuls: 1 cyc/row AND the HAM clock-gate warms the PE to 2.4 GHz
# (fp32-mode matmuls were observed to run at 1.2 GHz for the whole kernel).
BF16 = mybir.dt.bfloat16
F32R = mybir.dt.float32r
MM_DT = BF16

N_KT = S // 128          # 32 k-tiles
N_QH = 2                 # q halves per core
QH = SQ // N_QH          # 1024
N_QT = SQ // 128         # 16 q-tiles (for transposing Q)


def _mm(t):
    """View an f32 AP with the matmul streaming dtype."""
    return t.bitcast(MM_DT) if MM_DT is not F32 else t


def _emit(ctx: ExitStack, tc: "tile.TileContext", q_ext, k_ext, v_ext, out_ext):
    nc = tc.nc

    const_pool = ctx.enter_context(tc.tile_pool(name="const", bufs=1))
    qin_pool = ctx.enter_context(tc.tile_pool(name="qin", bufs=4))
    kin_pool = ctx.enter_context(tc.tile_pool(name="kin", bufs=4))
    qt_pool = ctx.enter_context(tc.tile_pool(name="qt", bufs=1))
    kt_pool = ctx.enter_context(tc.tile_pool(name="kt", bufs=1))
    v_pool = ctx.enter_context(tc.tile_pool(name="vv", bufs=1))
    probs_pool = ctx.enter_context(tc.tile_pool(name="probs", bufs=3))
    outsb_pool = ctx.enter_context(tc.tile_pool(name="outsb", bufs=2))
    res_pool = ctx.enter_context(tc.tile_pool(name="res", bufs=3))
    rcol_pool = ctx.enter_context(tc.tile_pool(name="rcol", bufs=3))

    # PSUM: 8 banks x 2KiB. tpsum 2x[128,128] = 2 banks; scores 2x[128,1024]
    # = 4 banks; outT' accumulator [65,1024] = 2 banks. Total 8.
    tpsum_pool = ctx.enter_context(tc.tile_pool(name="tpsum", bufs=2, space="PSUM"))
    spsum_pool = ctx.enter_context(tc.tile_pool(name="spsum", bufs=2, space="PSUM"))
    opsum_pool = ctx.enter_context(tc.tile_pool(name="opsum", bufs=1, space="PSUM"))

    identity = const_pool.tile([128, 128], F32)
    masks.make_identity(nc, identity[:])
    identity_mm = const_pool.tile([128, 128], MM_DT)
    masks.make_identity(nc, identity_mm[:])

    # --- Q^T [64, SQ] ---
    qt = qt_pool.tile([64, SQ], MM_DT)
    for t in range(N_QT):
        qin = qin_pool.tile([128, D], F32)
        nc.sync.dma_start(qin[:], q_ext[t * 128:(t + 1) * 128, :])
        qc = qin_pool.tile([128, D], MM_DT, tag="qc")
        nc.vector.tensor_copy(qc[:], qin[:])
        pt = tpsum_pool.tile([128, 128], MM_DT, tag="tp")
        nc.tensor.transpose(pt[:64, :128], qc[:], identity_mm[:])
        nc.vector.tensor_copy(qt[:, t * 128:(t + 1) * 128], pt[:64, :128])

    # --- K^T [64, S] ---
    kt = kt_pool.tile([64, S], MM_DT)
    for i in range(N_KT):
        kin = kin_pool.tile([128, D], F32)
        nc.sync.dma_start(kin[:], k_ext[i * 128:(i + 1) * 128, :])
        kc = kin_pool.tile([128, D], MM_DT, tag="kc")
        nc.vector.tensor_copy(kc[:], kin[:])
        pt = tpsum_pool.tile([128, 128], MM_DT, tag="tp")
        nc.tensor.transpose(pt[:64, :128], kc[:], identity_mm[:])
        nc.vector.tensor_copy(kt[:, i * 128:(i + 1) * 128], pt[:64, :128])

    # --- V' tiles [128, 65] with ones column ---
    # DMA lands in an f32 staging tile; a DVE copy performs the f32->f32r
    # rounding walrus requires for matmul operands.
    v_tiles = []
    for i in range(N_KT):
        vstage = kin_pool.tile([128, 65], F32, tag="vstage")
        nc.gpsimd.memset(vstage[:, 64:65], 1.0)
        nc.sync.dma_start(vstage[:, 0:64], v_ext[i * 128:(i + 1) * 128, :])
        vt = v_pool.tile([128, 65], MM_DT, tag=f"v{i}")
        nc.vector.tensor_copy(vt[:, :], vstage[:, :])
        v_tiles.append(vt)

    # --- main loop ---
    for h in range(N_QH):
        q0 = h * QH
        outp = opsum_pool.tile([65, QH], F32)
        for i in range(N_KT):
            sp = spsum_pool.tile([128, QH], F32)
            for j in range(QH // 512):
                nc.tensor.matmul(
                    sp[:, j * 512:(j + 1) * 512],
                    kt[:, i * 128:(i + 1) * 128],
                    qt[:, q0 + j * 512:q0 + (j + 1) * 512],
                    start=True,
                    stop=True,
                )
            pb = probs_pool.tile([128, QH], MM_DT)
            nc.scalar.activation(
                pb[:], sp[:], mybir.ActivationFunctionType.Exp, scale=float(SCALE)
            )
            for j in range(QH // 512):
                nc.tensor.matmul(
                    outp[:, j * 512:(j + 1) * 512],
                    v_tiles[i][:, :],
                    pb[:, j * 512:(j + 1) * 512],
                    start=(i == 0),
                    stop=(i == N_KT - 1),
                )
        # epilogue for this q-half
        osb = outsb_pool.tile([65, QH], F32)
        nc.vector.tensor_copy(osb[:], outp[:])
        for j in range(QH // 128):
            pt = tpsum_pool.tile([128, 128], F32, tag="tp")
            nc.tensor.transpose(
                pt[:128, :65], osb[:, j * 128:(j + 1) * 128], identity[:65, :65]
            )
            rcol = rcol_pool.tile([128, 1], F32)
            nc.vector.reciprocal(rcol[:], pt[:, 64:65])
            res = res_pool.tile([128, 64], F32)
            nc.vector.tensor_scalar_mul(res[:], pt[:, 0:64], rcol[:])
            nc.sync.dma_start(
                out_ext[q0 + j * 128:q0 + (j + 1) * 128, :], res[:]
            )


_CACHED_NC = None
LAST_RESULT = None


def _build():
    global _CACHED_NC
    if _CACHED_NC is not None:
        return _CACHED_NC
    nc = bacc.Bacc("TRN2", target_bir_lowering=False, debug=False, num_devices=N_CORES)
    q_ext = nc.declare_dram_parameter("Q", [SQ, D], F32, isOutput=False)
    k_ext = nc.declare_dram_parameter("K", [S, D], F32, isOutput=False)
    v_ext = nc.declare_dram_parameter("V", [S, D], F32, isOutput=False)
    out_ext = nc.declare_dram_parameter("out", [SQ, D], F32, isOutput=True)
    with tile.TileContext(nc) as tc:
        with ExitStack() as ctx:
            _emit(ctx, tc, q_ext.ap(), k_ext.ap(), v_ext.ap(), out_ext.ap())
    nc.compile()
    _CACHED_NC = nc
    return nc


def kernel(Q: np.ndarray, K: np.ndarray, V: np.ndarray) -> np.ndarray:
    global LAST_RESULT
    Q = np.ascontiguousarray(np.asarray(Q, dtype=np.float32))
    K = np.ascontiguousarray(np.asarray(K, dtype=np.float32))
    V = np.ascontiguousarray(np.asarray(V, dtype=np.float32))
    assert Q.shape == (B, S, D) and K.shape == (B, S, D) and V.shape == (B, S, D)

    nc = _build()
    in_maps = []
    for c in range(N_CORES):
        b, h = divmod(c, N_QH)
        in_maps.append(
            {
                "Q": np.ascontiguousarray(Q[b, h * SQ:(h + 1) * SQ, :]),
                "K": K[b],
                "V": V[b],
            }
        )
    trace = bool(os.environ.get("KERNEL_TRACE"))
    LAST_RESULT = run_bass_kernel_spmd(
        nc, in_maps, core_ids=list(range(N_CORES)), trace=trace
    )
    out = np.empty((B, S, D), dtype=np.float32)
    for c in range(N_CORES):
        b, h = divmod(c, N_QH)
        out[b, h * SQ:(h + 1) * SQ, :] = LAST_RESULT.results[c]["out"]
    return out


# revision 7
# speedup vs baseline: 1.6075x; 1.2815x over previous
"""Distributed attention head kernel for 8 TRN2 NeuronCores.

Problem: Q,K,V [4,4096,64] f32; out = softmax(Q K^T / 8) V  per batch.

Sharding (data parallel, zero collectives): core c handles batch b=c//2,
query half h=c%2 -> per-core Q [2048,64], K/V full [4096,64] for that batch.

Per-core algorithm (transposed-scores formulation so probs never need an
on-chip transpose before the PV matmul):
  - QT [64,2048], KT [64,4096] built with TensorE transposes.
  - V loaded as 32 tiles [128,65] with a ones column (65th col) so the
    softmax denominator falls out of the PV matmul for free.
  - for each q-half (1024 q) x 32 k-tiles:
      scoresT[k=128, q=1024] = KT_tile^T-matmul in PSUM (f32r, 2 MMs N=512)
      probsT = exp(scale * scoresT)  (ScalarE, 1 call, PSUM->SBUF)
      outT'[65, 1024] += V'_tile^T @ probsT   (accumulate in PSUM)
  - epilogue: reciprocal of denominator row, TensorE-transpose back to
    [128,65], per-partition scalar multiply, DMA out.
"""

import os
import sys
from contextlib import ExitStack

import numpy as np

sys.path.insert(0, "/opt/trn_rl_repo")

import concourse.bass as bass  # noqa: E402
import concourse.tile as tile  # noqa: E402
from concourse import bacc, masks, mybir  # noqa: E402
from concourse.bass_utils import run_bass_kernel_spmd  # noqa: E402

B, S, D = 4, 4096, 64
N_CORES = 8
SQ = B * S // N_CORES  # 2048 queries per core
SCALE = 1.0 / np.sqrt(np.float32(D))  # 0.125

F32 = mybir.dt.float32
# bf16 matmuls: 1 cyc/row AND the HAM clock-gate warms the PE to 2.4 GHz
# (fp32-mode matmuls were observed to run at 1.2 GHz for the whole kernel).
BF16 = mybir.dt.bfloat16
F32R = mybir.dt.float32r
MM_DT = BF16

N_KT = S // 128          # 32 k-tiles
N_QH = 2                 # q halves per core
QH = SQ // N_QH          # 1024
N_QT = SQ // 128         # 16 q-tiles (for transposing Q)


def _mm(t):
    """View an f32 AP with the matmul streaming dtype."""
    return t.bitcast(MM_DT) if MM_DT is not F32 else t


def _emit(ctx: ExitStack, tc: "tile.TileContext", q_ext, k_ext, v_ext, out_ext):
    nc = tc.nc

    const_pool = ctx.enter_context(tc.tile_pool(name="const", bufs=1))
    qin_pool = ctx.enter_context(tc.tile_pool(name="qin", bufs=4))
    kin_pool = ctx.enter_context(tc.tile_pool(name="kin", bufs=4))
    qt_pool = ctx.enter_context(tc.tile_pool(name="qt", bufs=1))
    kt_pool = ctx.enter_context(tc.tile_pool(name="kt", bufs=1))
    v_pool = ctx.enter_context(tc.tile_pool(name="vv", bufs=1))
    probs_pool = ctx.enter_context(tc.tile_pool(name="probs", bufs=3))
    outsb_pool = ctx.enter_context(tc.tile_pool(name="outsb", bufs=2))
    res_pool = ctx.enter_context(tc.tile_pool(name="res", bufs=3))
    rcol_pool = ctx.enter_context(tc.tile_pool(name="rcol", bufs=3))

    # PSUM: 8 banks x 2KiB. tpsum 2x[128,128] = 2 banks; scores 2x[128,1024]
    # = 4 banks; outT' accumulator [65,1024] = 2 banks. Total 8.
    tpsum_pool = ctx.enter_context(tc.tile_pool(name="tpsum", bufs=2, space="PSUM"))
    spsum_pool = ctx.enter_context(tc.tile_pool(name="spsum", bufs=2, space="PSUM"))
    opsum_pool = ctx.enter_context(tc.tile_pool(name="opsum", bufs=1, space="PSUM"))

    identity = const_pool.tile([128, 128], F32)
    masks.make_identity(nc, identity[:])
    identity_mm = const_pool.tile([128, 128], MM_DT)
    masks.make_identity(nc, identity_mm[:])

    # --- Q^T padded to [128, SQ]: rows 64-127 zero so the scores matmuls
    # run a full 128-row contraction (64-row MMs never warm the PE clock
    # gate -- measured 427ns/MM forever vs 216ns warm) ---
    qt = qt_pool.tile([128, SQ], MM_DT)
    nc.vector.memset(qt[64:128, :], 0.0)
    for t in range(N_QT):
        qin = qin_pool.tile([128, D], F32)
        nc.sync.dma_start(qin[:], q_ext[t * 128:(t + 1) * 128, :])
        qc = qin_pool.tile([128, D], MM_DT, tag="qc")
        nc.vector.tensor_copy(qc[:], qin[:])
        pt = tpsum_pool.tile([128, 128], MM_DT, tag="tp")
        nc.tensor.transpose(pt[:64, :128], qc[:], identity_mm[:])
        nc.vector.tensor_copy(qt[0:64, t * 128:(t + 1) * 128], pt[:64, :128])

    # --- K^T padded to [128, S], rows 64-127 zero ---
    kt = kt_pool.tile([128, S], MM_DT)
    nc.vector.memset(kt[64:128, :], 0.0)
    for i in range(N_KT):
        kin = kin_pool.tile([128, D], F32)
        nc.sync.dma_start(kin[:], k_ext[i * 128:(i + 1) * 128, :])
        kc = kin_pool.tile([128, D], MM_DT, tag="kc")
        nc.vector.tensor_copy(kc[:], kin[:])
        pt = tpsum_pool.tile([128, 128], MM_DT, tag="tp")
        nc.tensor.transpose(pt[:64, :128], kc[:], identity_mm[:])
        nc.vector.tensor_copy(kt[0:64, i * 128:(i + 1) * 128], pt[:64, :128])

    # --- V' tiles [128, 65] with ones column ---
    # DMA lands in an f32 staging tile; a DVE copy performs the f32->f32r
    # rounding walrus requires for matmul operands.
    v_tiles = []
    for i in range(N_KT):
        vstage = kin_pool.tile([128, 65], F32, tag="vstage")
        nc.gpsimd.memset(vstage[:, 64:65], 1.0)
        nc.sync.dma_start(vstage[:, 0:64], v_ext[i * 128:(i + 1) * 128, :])
        vt = v_pool.tile([128, 65], MM_DT, tag=f"v{i}")
        nc.vector.tensor_copy(vt[:, :], vstage[:, :])
        v_tiles.append(vt)

    # --- main loop ---
    for h in range(N_QH):
        q0 = h * QH
        outp = opsum_pool.tile([65, QH], F32)
        for i in range(N_KT):
            sp = spsum_pool.tile([128, QH], F32)
            for j in range(QH // 512):
                nc.tensor.matmul(
                    sp[:, j * 512:(j + 1) * 512],
                    kt[:, i * 128:(i + 1) * 128],
                    qt[:, q0 + j * 512:q0 + (j + 1) * 512],
                    start=True,
                    stop=True,
                )
            pb = probs_pool.tile([128, QH], MM_DT)
            nc.scalar.activation(
                pb[:], sp[:], mybir.ActivationFunctionType.Exp, scale=float(SCALE)
            )
            for j in range(QH // 512):
                nc.tensor.matmul(
                    outp[:, j * 512:(j + 1) * 512],
                    v_tiles[i][:, :],
                    pb[:, j * 512:(j + 1) * 512],
                    start=(i == 0),
                    stop=(i == N_KT - 1),
                )
        # epilogue for this q-half
        osb = outsb_pool.tile([65, QH], F32)
        nc.vector.tensor_copy(osb[:], outp[:])
        for j in range(QH // 128):
            pt = tpsum_pool.tile([128, 128], F32, tag="tp")
            nc.tensor.transpose(
                pt[:128, :65], osb[:, j * 128:(j + 1) * 128], identity[:65, :65]
            )
            rcol = rcol_pool.tile([128, 1], F32)
            nc.vector.reciprocal(rcol[:], pt[:, 64:65])
            res = res_pool.tile([128, 64], F32)
            nc.vector.tensor_scalar_mul(res[:], pt[:, 0:64], rcol[:])
            nc.sync.dma_start(
                out_ext[q0 + j * 128:q0 + (j + 1) * 128, :], res[:]
            )


_CACHED_NC = None
LAST_RESULT = None


def _build():
    global _CACHED_NC
    if _CACHED_NC is not None:
        return _CACHED_NC
    nc = bacc.Bacc("TRN2", target_bir_lowering=False, debug=False, num_devices=N_CORES)
    q_ext = nc.declare_dram_parameter("Q", [SQ, D], F32, isOutput=False)
    k_ext = nc.declare_dram_parameter("K", [S, D], F32, isOutput=False)
    v_ext = nc.declare_dram_parameter("V", [S, D], F32, isOutput=False)
    out_ext = nc.declare_dram_parameter("out", [SQ, D], F32, isOutput=True)
    with tile.TileContext(nc) as tc:
        with ExitStack() as ctx:
            _emit(ctx, tc, q_ext.ap(), k_ext.ap(), v_ext.ap(), out_ext.ap())
    nc.compile()
    _CACHED_NC = nc
    return nc


def kernel(Q: np.ndarray, K: np.ndarray, V: np.ndarray) -> np.ndarray:
    global LAST_RESULT
    Q = np.ascontiguousarray(np.asarray(Q, dtype=np.float32))
    K = np.ascontiguousarray(np.asarray(K, dtype=np.float32))
    V = np.ascontiguousarray(np.asarray(V, dtype=np.float32))
    assert Q.shape == (B, S, D) and K.shape == (B, S, D) and V.shape == (B, S, D)

    nc = _build()
    in_maps = []
    for c in range(N_CORES):
        b, h = divmod(c, N_QH)
        in_maps.append(
            {
                "Q": np.ascontiguousarray(Q[b, h * SQ:(h + 1) * SQ, :]),
                "K": K[b],
                "V": V[b],
            }
        )
    trace = bool(os.environ.get("KERNEL_TRACE"))
    LAST_RESULT = run_bass_kernel_spmd(
        nc, in_maps, core_ids=list(range(N_CORES)), trace=trace
    )
    out = np.empty((B, S, D), dtype=np.float32)
    for c in range(N_CORES):
        b, h = divmod(c, N_QH)
        out[b, h * SQ:(h + 1) * SQ, :] = LAST_RESULT.results[c]["out"]
    return out
